# revision 23
# baseline (speedup 1.0000x reference)
"""Trainium2 Bass kernel for nn_CAMEncoder3 (2-layer GATv2 GNN encoder).

Self-contained: kernel(**inputs) -> np.ndarray [50000, 192] float32.

V2: node-range sharding over 8 NeuronCores. Per conv pass, edges bucketed by
destination core/128-node block, lo/hi src halves for int16 dma_gather range.
Only the SOURCE side is DMA-gathered (one gather per edge); the destination
transform xr[dst] is reconstructed on the PE via a one-hot matmul from an
SBUF-resident per-shard xr table, and the e = xl[src]+xr[dst] sum is also
accumulated on the PE (identity matmul) so the Vector engine never touches
PSUM for the add. One-hot matrices (both orientations) are built with
is_equal against step-1 constant iota tiles so the DVE runs in 2x packed
mode. Gather descriptor generation (the GpSimd bottleneck) is trimmed with
per-core runtime valid-edge counts (num_idxs_reg) so padding lanes are
skipped. j1/j2 and m1/m2 share edge streams (identical schedules).
"""
import os
import sys

sys.path.insert(0, '/opt/trn_rl_repo')

import numpy as np
import ml_dtypes
import concourse.bacc as bacc
import concourse.mybir as mybir
import concourse.tile as tile
from concourse.bass_utils import run_bass_kernel_spmd
from concourse.library_config import mlp

bf16 = mybir.dt.bfloat16
f32 = mybir.dt.float32
i16 = mybir.dt.int16
i32 = mybir.dt.int32
AT = mybir.AluOpType
AF = mybir.ActivationFunctionType

CH = 8          # subtiles per gather chunk (NIDX = 1024)
QUAD = 4        # subtiles per PSUM e-accumulation group
PADLOC = 200.0  # dst_local sentinel for pad edges
LGCLAMP = 30.0  # logit clamp before exp (stale-lane safety)


def _wrap16(a):
    """int16 stream -> dma_gather wrapped layout [128, n/16]."""
    n = a.shape[0]
    w = np.zeros((16, (n + 15) // 16), np.int16)
    w[np.arange(n) % 16, np.arange(n) // 16] = a
    return np.tile(w, (8, 1))


def preprocess_conv(src, dst, ncores, npc, npad, split):
    """Bucket edges per core/block/half, pad to subtiles, build shared
    schedule and per-core streams.

    Returns sched, ns_lo, ns_hi, per-core streams (src16, dloc col layout,
    dlocR row layout), and per-core per-(block,half) edge counts."""
    nb = npad // 128
    rows = (src // npc) * npad + (src % npc)  # AG table row of src
    percore = []
    for c in range(ncores):
        sel = (dst >= c * npc) & (dst < (c + 1) * npc)
        r, d = rows[sel], dst[sel] - c * npc
        blk = d // 128
        lo = r < split
        blocks = []
        for b in range(nb):
            m = blk == b
            blocks.append((r[m & lo], d[m & lo], r[m & ~lo], d[m & ~lo]))
        percore.append(blocks)
    sched = []
    for b in range(nb):
        nlo = max(len(percore[c][b][0]) for c in range(ncores))
        nhi = max(len(percore[c][b][2]) for c in range(ncores))
        nlo = (nlo + 127) // 128
        nhi = (nhi + 127) // 128
        if nlo + nhi == 0:
            nlo = 1
        sched.append((nlo, nhi))
    ns_lo = sum(s[0] for s in sched)
    ns_hi = sum(s[1] for s in sched)
    ns = ns_lo + ns_hi
    out = []
    for c in range(ncores):
        s16 = np.zeros(ns * 128, np.int64)
        s16n = np.full(ns * 128, -1, np.int64)  # -1 pads: gen skipped
        dl = np.full(ns * 128, PADLOC, np.float32)
        olo, ohi = 0, ns_lo
        for b in range(nb):
            rl, dloc_l, rh, dloc_h = percore[c][b]
            nlo, nhi = sched[b]
            e0 = olo * 128
            s16[e0:e0 + len(rl)] = rl
            s16n[e0:e0 + len(rl)] = rl
            dl[e0:e0 + len(rl)] = dloc_l % 128
            olo += nlo
            e0 = ohi * 128
            s16[e0:e0 + len(rh)] = rh - split
            s16n[e0:e0 + len(rh)] = rh - split
            dl[e0:e0 + len(rh)] = dloc_h % 128
            ohi += nhi
        dloc_col = dl.reshape(ns, 128).T.astype(ml_dtypes.bfloat16)
        out.append({
            "src16": _wrap16(s16.astype(np.int16)),
            "src16n": _wrap16(s16n.astype(np.int16)),
            "dloc": np.ascontiguousarray(dloc_col),
            "dlocR": dl[None, :].astype(ml_dtypes.bfloat16),
        })
    return sched, ns_lo, ns_hi, out


def chunk_plan(sched, ns_lo):
    """Block-aligned chunking: chunks never span dst blocks or halves.
    Returns cdef {(half,cid): (o0, nk, b)}, cmap {(half,ordinal):
    (cid, slot)}."""
    cmap, cdef = {}, {}
    for half in ("lo", "hi"):
        o, cid = 0, 0
        for b, (nlo, nhi) in enumerate(sched):
            n = nlo if half == "lo" else nhi
            for s0 in range(0, n, CH):
                nk = min(CH, n - s0)
                cdef[(half, cid)] = (o + s0, nk, b)
                for j in range(nk):
                    cmap[(half, o + s0 + j)] = (cid, j)
                cid += 1
            o += n
    return cdef, cmap


def aug_w(W, b):
    """[din, HC],[HC] -> bf16 [din+1, HC] with bias row appended."""
    return np.vstack([W, b[None, :]]).astype(ml_dtypes.bfloat16)


def build_program(ncores, npc, npad, split, scheds, plans):
    """Build the full Bass program.
    scheds: dict edgeset->(sched, ns_lo, ns_hi); plans: edgeset->(cdef,cmap,nch)
    Returns nc."""
    nb = npad // 128
    npadg = ncores * npad
    nc = bacc.Bacc("TRN2", target_bir_lowering=False, debug=False,
                   num_devices=ncores, num_swdge_queues=4)

    D1, D2 = 128, 256   # HC per layer
    xbt = nc.dram_tensor("xbt", [65, npad], bf16, kind="ExternalInput")
    iotin = nc.dram_tensor("iot", [1, 128], bf16, kind="ExternalInput")
    identin = nc.dram_tensor("ident", [128, 128], bf16, kind="ExternalInput")
    iotPin = nc.dram_tensor("iotP", [128, CH * 128], bf16,
                            kind="ExternalInput")
    epsin = nc.dram_tensor("epsin", [1, 2], f32, kind="ExternalInput")

    # shared per-edge-set streams (j: job edges, m: mac edges). The j set
    # has a pad-0 variant (conv j1 runs first, pool buffers uninitialized)
    # and a pad(-1) variant (descriptor gen skipped) for conv j2.
    estreams = {}
    for es in ("j", "m"):
        ns = scheds[es][1] + scheds[es][2]
        estreams[es] = {
            "dlocR": nc.dram_tensor(f"{es}_dlocR", [1, ns * 128], bf16,
                                    kind="ExternalInput"),
            "dloc": nc.dram_tensor(f"{es}_dloc", [128, ns], bf16,
                                   kind="ExternalInput"),
        }
    estreams["j"]["src16_a"] = nc.dram_tensor(
        "j_src16_a", [128, (scheds["j"][1] + scheds["j"][2]) * 8], i16,
        kind="ExternalInput")
    estreams["j"]["src16_b"] = nc.dram_tensor(
        "j_src16_b", [128, (scheds["j"][1] + scheds["j"][2]) * 8], i16,
        kind="ExternalInput")
    estreams["m"]["src16_b"] = nc.dram_tensor(
        "m_src16_b", [128, (scheds["m"][1] + scheds["m"][2]) * 8], i16,
        kind="ExternalInput")

    convs = {}
    for p, hc in [("j1", D1), ("m1", D1), ("j2", D2), ("m2", D2)]:
        t = {}
        t["att"] = nc.dram_tensor(f"{p}_att", [1, hc], bf16,
                                  kind="ExternalInput")
        t["wla"] = nc.dram_tensor(f"{p}_wla", [65, hc], bf16,
                                  kind="ExternalInput")
        t["wra"] = nc.dram_tensor(f"{p}_wra", [65, hc], bf16,
                                  kind="ExternalInput")
        if p in ("j2", "m2"):
            t["wlb"] = nc.dram_tensor(f"{p}_wlb", [128, hc], bf16,
                                      kind="ExternalInput")
            t["wrb"] = nc.dram_tensor(f"{p}_wrb", [128, hc], bf16,
                                      kind="ExternalInput")
            t["bias"] = nc.dram_tensor(f"{p}_bias", [1, 128], f32,
                                       kind="ExternalInput")
        else:
            t["bias"] = nc.dram_tensor(f"{p}_bias", [1, 64], f32,
                                       kind="ExternalInput")
        convs[p] = t
    outp = nc.dram_tensor("outp", [npad, 128], f32, kind="ExternalOutput")

    qc = [0]  # gather queue rotator

    with tile.TileContext(nc) as tc:
        with (
            tc.tile_pool(name="res", bufs=1) as rp,
            tc.tile_pool(name="stream", bufs=1) as sp,
            tc.tile_pool(name="gat", bufs=4) as gp,
            tc.tile_pool(name="wrk", bufs=2) as wp,
            tc.tile_pool(name="epi", bufs=2) as ep,
            tc.tile_pool(name="tb", bufs=2) as tbp,
            tc.tile_pool(name="pacc", bufs=2, space="PSUM") as pacc,
            tc.tile_pool(name="peps", bufs=2, space="PSUM") as peps,
            tc.tile_pool(name="ptb", bufs=1, space="PSUM") as ptb,
            tc.tile_pool(name="dram", bufs=1, space="DRAM") as dp,
        ):
            nc.gpsimd.load_library(mlp)
            xbt_t = rp.tile([65, npad], bf16)
            nc.sync.dma_start(xbt_t[:], xbt[:])
            iota_t = rp.tile([128, 128], bf16)
            nc.sync.dma_start(iota_t[:], iotin[:].to_broadcast((128, 128)))
            ident_t = rp.tile([128, 128], bf16)
            nc.sync.dma_start(ident_t[:], identin[:])
            iotP_t = rp.tile([128, CH * 128], bf16)
            nc.sync.dma_start(iotP_t[:], iotPin[:])
            eps_t = rp.tile([128, 2], f32)
            nc.sync.dma_start(eps_t[:], epsin[:].to_broadcast((128, 2)))
            h1T = rp.tile([128, npad], bf16)
            stash = rp.tile([128, npad], bf16)

            def load_streams(es, variant):
                """Load shared streams for edge set es; returns dict."""
                ns = scheds[es][1] + scheds[es][2]
                st = estreams[es]
                s16_t = sp.tile([128, ns * 8], i16, tag=f"s16_{es}_{variant}")
                nc.sync.dma_start(s16_t[:], st[f"src16_{variant}"][:])
                dloc_t = sp.tile([128, ns], bf16, tag=f"dloc_{es}")
                nc.sync.dma_start(dloc_t[:], st["dloc"][:])
                return {"s16": s16_t, "dlocR": st["dlocR"], "dloc": dloc_t}

            def build_tables(p, hc, with_h1):
                """Own-shard XL (dram, AllGathered) + resident SBUF XR table."""
                t = convs[p]
                wla_t = rp.tile([65, hc], bf16, tag=f"w_{p}l")
                nc.sync.dma_start(wla_t[:], t["wla"][:])
                wra_t = rp.tile([65, hc], bf16, tag=f"w_{p}r")
                nc.sync.dma_start(wra_t[:], t["wra"][:])
                if with_h1:
                    wlb_t = rp.tile([128, hc], bf16, tag=f"w_{p}lb")
                    nc.sync.dma_start(wlb_t[:], t["wlb"][:])
                    wrb_t = rp.tile([128, hc], bf16, tag=f"w_{p}rb")
                    nc.sync.dma_start(wrb_t[:], t["wrb"][:])
                xl_sh = dp.tile([npad, hc], bf16, tag=f"xlsh_{p}")
                xl_full = dp.tile([npadg, hc], bf16, tag=f"xlf_{p}")
                # resident xr: [128, nb, D2] (L1 uses low hc cols)
                xr_t = rp.tile([128, nb, D2], bf16, tag=f"xr_{p[0]}")
                for side in (0, 1):
                    wa = wla_t if side == 0 else wra_t
                    for jt in range(nb):
                        ps = ptb.tile([128, hc], f32, space="PSUM", tag="tb")
                        nc.tensor.matmul(ps[:], lhsT=xbt_t[:, jt * 128:(jt + 1) * 128],
                                         rhs=wa[:], start=True, stop=not with_h1)
                        if with_h1:
                            wb = wlb_t if side == 0 else wrb_t
                            nc.tensor.matmul(ps[:],
                                             lhsT=h1T[:, jt * 128:(jt + 1) * 128],
                                             rhs=wb[:], start=False, stop=True)
                        if side == 0:
                            sb = tbp.tile([128, hc], bf16, tag="tbsb")
                            nc.scalar.copy(sb[:], ps[:])
                            nc.sync.dma_start(
                                xl_sh[jt * 128:(jt + 1) * 128, :], sb[:])
                        else:
                            nc.scalar.copy(xr_t[:, jt, 0:hc], ps[:])
                nc.gpsimd.collective_compute(
                    "AllGather", AT.bypass,
                    replica_groups=[list(range(ncores))],
                    ins=[xl_sh.opt()], outs=[xl_full.opt()])
                convs[p]["xlf"] = xl_full
                convs[p]["xr"] = xr_t

            def conv_pass(p, hc, epi, variant="b"):
                es = p[0]
                sched, ns_lo, ns_hi = scheds[es]
                cdef, cmap = plans[es]
                t = convs[p]
                xl_full, xr_t = t["xlf"], t["xr"]
                strm = load_streams(es, variant)
                s16_t = strm["s16"]
                dlocR, dlocC = strm["dlocR"], strm["dloc"]
                chc = hc // 2
                att_t = rp.tile([128, hc], bf16, tag=f"att_{p}")
                nc.sync.dma_start(att_t[:], t["att"][:].to_broadcast((128, hc)))
                bw = 64 if p in ("j1", "m1") else 128
                bias_t = rp.tile([128, bw], f32, tag=f"bias_{p}")
                nc.sync.dma_start(bias_t[:], t["bias"][:].to_broadcast((128, bw)))

                xl_lo = xl_full[0:split, :]
                xl_hi = xl_full[split:npadg, :]
                half_off = {"lo": 0, "hi": ns_lo}
                chunks = {}

                def get_chunk(half, k):
                    key = (half, k)
                    if key in chunks:
                        return chunks[key]
                    o0, nk, b = cdef[key]
                    g0 = half_off[half] + o0  # global subtile offset
                    nidx = nk * 128
                    gt = gp.tile([128, CH, hc], bf16, tag="gsrc")
                    nc.gpsimd.dma_gather(
                        gt[:, 0:nk, :], xl_lo if half == "lo" else xl_hi,
                        s16_t[:, g0 * 8:(g0 + nk) * 8], nidx, nidx, hc,
                        queue_num=qc[0] % 4)
                    qc[0] += 1
                    dbc = gp.tile([128, CH * 128], bf16, tag="dbc", bufs=3)
                    nc.sync.dma_start(
                        dbc[:, 0:nidx],
                        dlocR[0:1, g0 * 128:(g0 + nk) * 128]
                        .to_broadcast((128, nidx)))
                    oh = gp.tile([128, CH, 128], bf16, tag="oh")
                    nc.vector.tensor_tensor(
                        out=oh[:, 0:nk, :],
                        in0=dlocC[:, g0:g0 + nk]
                        .rearrange("p (a b) -> p a b", b=1)
                        .to_broadcast((128, nk, 128)),
                        in1=iota_t[:].rearrange("p (a c) -> p a c", a=1)
                        .to_broadcast((128, nk, 128)), op=AT.is_equal)
                    ohT = gp.tile([128, CH * 128], bf16, tag="ohT")
                    nc.vector.tensor_tensor(
                        out=ohT[:, 0:nidx], in0=iotP_t[:, 0:nidx],
                        in1=dbc[:, 0:nidx], op=AT.is_equal)
                    ea = gp.tile([128, CH, hc], bf16, tag="ea", bufs=2)
                    for q0 in range(0, nk, QUAD):
                        qn = min(QUAD, nk - q0)
                        eps = peps.tile([128, QUAD, hc], f32, space="PSUM",
                                        tag="eps")
                        for s in range(q0, q0 + qn):
                            nc.tensor.matmul(
                                eps[:, s - q0, :],
                                lhsT=ohT[:, s * 128:(s + 1) * 128],
                                rhs=xr_t[:, b, 0:hc], start=True, stop=False)
                            nc.tensor.matmul(
                                eps[:, s - q0, :], lhsT=ident_t[:],
                                rhs=gt[:, s, :], start=False, stop=True)
                        nc.scalar.activation(
                            ea[:, q0:q0 + qn, :], eps[:, 0:qn, :],
                            AF.Prelu, alpha=0.15)
                    es_t = wp.tile([128, CH, hc], bf16, tag="es")
                    nc.vector.tensor_tensor(
                        out=es_t[:, 0:nk, :], in0=ea[:, 0:nk, :],
                        in1=att_t[:].rearrange("p (a c) -> p a c", a=1)
                        .to_broadcast((128, nk, hc)), op=AT.mult)
                    hs_t = wp.tile([128, CH * 2, chc // 2], bf16, tag="hs")
                    ev = es_t[:, 0:nk, :].rearrange(
                        "p a (h g c) -> p (a h) g c", h=2, g=2)
                    nc.vector.tensor_tensor(
                        out=hs_t[:, 0:nk * 2, :],
                        in0=ev[:, :, 0, :], in1=ev[:, :, 1, :], op=AT.add)
                    lg = wp.tile([128, CH * 2], f32, tag="lg")
                    nc.vector.tensor_reduce(
                        out=lg[:, 0:nk * 2],
                        in_=hs_t[:, 0:nk * 2, :],
                        axis=mybir.AxisListType.X, op=AT.add)
                    gtw = wp.tile([128, CH, hc + 2], bf16, tag="gtw")
                    nc.scalar.activation(
                        gtw[:, 0:nk, hc:hc + 2],
                        lg[:, 0:nk * 2].rearrange("p (a h) -> p a h", h=2),
                        AF.Exp)
                    nc.vector.tensor_tensor(
                        out=gtw[:, 0:nk, 0:hc].rearrange(
                            "p a (h c) -> p a h c", h=2),
                        in0=gt[:, 0:nk, :].rearrange(
                            "p a (h c) -> p a h c", h=2),
                        in1=gtw[:, 0:nk, hc:hc + 2]
                        .rearrange("p a (h o) -> p a h o", o=1)
                        .to_broadcast((128, nk, 2, chc)), op=AT.mult)
                    ck = {"gtw": gtw, "oh": oh}
                    chunks[key] = ck
                    return ck

                olo, ohi = 0, 0
                for b in range(nb):
                    nlo, nhi = sched[b]
                    subs = ([("lo", olo + i) for i in range(nlo)]
                            + [("hi", ohi + i) for i in range(nhi)])
                    olo += nlo
                    ohi += nhi
                    acc = pacc.tile([128, hc + 2], f32, space="PSUM", tag="acc")
                    n = len(subs)
                    for si, (half, o) in enumerate(subs):
                        cid, s = cmap[(half, o)]
                        ck = get_chunk(half, cid)
                        nc.tensor.matmul(acc[:, 0:hc + 2],
                                         lhsT=ck["oh"][:, s, :],
                                         rhs=ck["gtw"][:, s, :],
                                         start=(si == 0), stop=(si == n - 1))
                    epi(b, acc, bias_t)

            def epi_l1(rowoff):
                def f(b, acc, bias_t):
                    cp = ep.tile([128, 130], f32, tag="cp")
                    nc.scalar.copy(cp[:], acc[:])
                    den = ep.tile([128, 2], f32, tag="den")
                    nc.vector.tensor_tensor(out=den[:], in0=cp[:, 128:130],
                                            in1=eps_t[:], op=AT.add)
                    nc.vector.reciprocal(den[:], den[:])
                    xt = ep.tile([128, 2, 64], f32, tag="xt")
                    nc.vector.tensor_tensor(
                        out=xt[:],
                        in0=cp[:, 0:128].rearrange("p (h c) -> p h c", h=2),
                        in1=den[:].rearrange("p (h o) -> p h o", o=1)
                        .to_broadcast((128, 2, 64)), op=AT.mult)
                    st = ep.tile([128, 64], f32, tag="st")
                    nc.vector.tensor_tensor(out=st[:], in0=xt[:, 0, :],
                                            in1=xt[:, 1, :], op=AT.add)
                    nc.vector.tensor_tensor(out=st[:], in0=st[:], in1=bias_t[:],
                                            op=AT.add)
                    hb = ep.tile([128, 64], bf16, tag="hb")
                    nc.scalar.activation(hb[:], st[:], AF.Relu, scale=0.5)
                    tp = ptb.tile([64, 128], bf16, space="PSUM", tag="tr")
                    nc.tensor.transpose(tp[:], hb[:], ident_t[:])
                    nc.vector.tensor_copy(
                        h1T[rowoff:rowoff + 64, b * 128:(b + 1) * 128], tp[:])
                return f

            def epi_l2(is_j):
                def f(b, acc, bias_t):
                    cp = ep.tile([128, 258], f32, tag="cp2")
                    nc.scalar.copy(cp[:], acc[:])
                    den = ep.tile([128, 2], f32, tag="den")
                    nc.vector.tensor_tensor(out=den[:], in0=cp[:, 256:258],
                                            in1=eps_t[:], op=AT.add)
                    nc.vector.reciprocal(den[:], den[:])
                    xt = ep.tile([128, 2, 128], f32, tag="xt2")
                    nc.vector.tensor_tensor(
                        out=xt[:],
                        in0=cp[:, 0:256].rearrange("p (h c) -> p h c", h=2),
                        in1=den[:].rearrange("p (h o) -> p h o", o=1)
                        .to_broadcast((128, 2, 128)), op=AT.mult)
                    st = ep.tile([128, 128], f32, tag="st2")
                    nc.vector.tensor_tensor(out=st[:], in0=xt[:, 0, :],
                                            in1=xt[:, 1, :], op=AT.add)
                    nc.vector.tensor_tensor(out=st[:], in0=st[:], in1=bias_t[:],
                                            op=AT.add)
                    if is_j:
                        nc.vector.tensor_copy(stash[:, b * 128:(b + 1) * 128],
                                              st[:])
                    else:
                        nc.vector.tensor_tensor(
                            out=st[:], in0=st[:],
                            in1=stash[:, b * 128:(b + 1) * 128], op=AT.add)
                        ob = ep.tile([128, 128], f32, tag="ob")
                        nc.scalar.activation(ob[:], st[:], AF.Relu, scale=0.25)
                        nc.sync.dma_start(outp[b * 128:(b + 1) * 128, :], ob[:])
                return f

            build_tables("j1", D1, False)
            conv_pass("j1", D1, epi_l1(0), variant="a")
            build_tables("m1", D1, False)
            conv_pass("m1", D1, epi_l1(64))
            build_tables("j2", D2, True)
            conv_pass("j2", D2, epi_l2(True), variant="a")
            build_tables("m2", D2, True)
            conv_pass("m2", D2, epi_l2(False))

    nc.compile()
    return nc


def run_full(inputs, N, ncores, split=32768, npc=None):
    """Host orchestration: preprocess, build, run, assemble output."""
    x = np.asarray(inputs["x"], np.float32)
    npc = npc or N // ncores
    npad = ((npc + 127) // 128) * 128
    IN = x.shape[1]

    def prep_edges(e):
        e = np.asarray(e)
        return e[0].astype(np.int64), e[1].astype(np.int64)

    js, jd = prep_edges(inputs["job_edges"])
    ms, md = prep_edges(inputs["mac_edges"])

    scheds, streams, plans = {}, {}, {}
    for es, (s, d) in [("j", (js, jd)), ("m", (ms, md))]:
        sch, nlo, nhi, pc = preprocess_conv(s, d, ncores, npc, npad, split)
        scheds[es] = (sch, nlo, nhi)
        streams[es] = pc
        plans[es] = chunk_plan(sch, nlo)

    nc = build_program(ncores, npc, npad, split, scheds, plans)

    # host-side constants
    xall = np.zeros((65, ncores * npad), np.float32)
    for c in range(ncores):
        n0 = c * npc
        w = min(npc, N - n0) if n0 < N else 0
        if w > 0:
            xall[0:IN, c * npad:c * npad + w] = x[n0:n0 + w].T
    xall[64, :] = 1.0
    xall = xall.astype(ml_dtypes.bfloat16)

    iot = np.arange(128, dtype=np.float32)[None, :].astype(ml_dtypes.bfloat16)
    ident = np.eye(128, dtype=np.float32).astype(ml_dtypes.bfloat16)
    iotP = np.tile(np.arange(128, dtype=np.float32)[:, None],
                   (1, CH * 128)).astype(ml_dtypes.bfloat16)

    consts = {}
    for p, key in [("j1", "jg1"), ("m1", "mg1"), ("j2", "jg2"), ("m2", "mg2")]:
        Wl = np.asarray(inputs[key + "_Wl"], np.float32)
        bl = np.asarray(inputs[key + "_bl"], np.float32)
        Wr = np.asarray(inputs[key + "_Wr"], np.float32)
        br = np.asarray(inputs[key + "_br"], np.float32)
        att = np.asarray(inputs[key + "_att"], np.float32)
        b = np.asarray(inputs[key + "_b"], np.float32)
        hc = Wl.shape[1]
        d = {}
        d[f"{p}_att"] = att.reshape(1, hc).astype(ml_dtypes.bfloat16)
        if p in ("j1", "m1"):
            d[f"{p}_wla"] = aug_w(Wl, bl)
            d[f"{p}_wra"] = aug_w(Wr, br)
            d[f"{p}_bias"] = (2.0 * b)[None, :].astype(np.float32)
        else:
            d[f"{p}_wla"] = aug_w(Wl[0:64], bl)
            d[f"{p}_wlb"] = Wl[64:192].astype(ml_dtypes.bfloat16)
            d[f"{p}_wra"] = aug_w(Wr[0:64], br)
            d[f"{p}_wrb"] = Wr[64:192].astype(ml_dtypes.bfloat16)
            d[f"{p}_bias"] = (2.0 * b)[None, :].astype(np.float32)
        consts.update(d)

    in_maps = []
    for c in range(ncores):
        m = {"xbt": xall[:, c * npad:(c + 1) * npad].copy(),
             "iot": iot, "ident": ident, "iotP": iotP,
             "epsin": np.full((1, 2), 1e-16, np.float32)}
        m.update(consts)
        stj, stm = streams["j"][c], streams["m"][c]
        m["j_src16_a"] = stj["src16"]
        m["j_src16_b"] = stj["src16"]
        m["j_dlocR"] = stj["dlocR"]
        m["j_dloc"] = stj["dloc"]
        m["m_src16_b"] = stm["src16"]
        m["m_dlocR"] = stm["dlocR"]
        m["m_dloc"] = stm["dloc"]
        in_maps.append(m)

    res = run_bass_kernel_spmd(nc, in_maps, list(range(ncores)),
                               trace=bool(int(os.environ.get(
                                   'GAT_TRACE', '0'))))
    parts = [res.results[c]["outp"][:min(npc, N - c * npc)]
             for c in range(ncores)]
    out = np.concatenate([x, np.vstack(parts)], axis=1).astype(np.float32)
    return out, res


def _install_profile_shim():
    """Optional: register the NTFF profiling hook (GAT_TRACE=1)."""
    try:
        import types
        import antenv
        if "antenv.axon_hooks" not in sys.modules:
            _store = {}
            m = types.ModuleType("antenv.axon_hooks")
            m.set_axon_ntff_profile_hook = lambda h: _store.__setitem__("h", h)
            m.get_axon_ntff_profile_hook = lambda: _store.get("h")
            sys.modules["antenv.axon_hooks"] = m
            antenv.axon_hooks = m
        from trn_agent_boot.trn_boot import _ntff_profile_via_ctypes
        sys.modules["antenv.axon_hooks"].set_axon_ntff_profile_hook(
            _ntff_profile_via_ctypes("/opt/axon/libaxon_pjrt.so"))
    except Exception:
        pass


LAST_RESULT = None


def kernel(**inputs):
    global LAST_RESULT
    if os.environ.get("GAT_TRACE", "0") == "1":
        _install_profile_shim()
    out, res = run_full(inputs, 50000, 8)
    LAST_RESULT = res
    return out


# revision 24
# speedup vs baseline: 1.0777x; 1.0777x over previous
"""Trainium2 Bass kernel for nn_CAMEncoder3 (2-layer GATv2 GNN encoder).

Self-contained: kernel(**inputs) -> np.ndarray [50000, 192] float32.

V2: node-range sharding over 8 NeuronCores. Per conv pass, edges bucketed by
destination core/128-node block, lo/hi src halves for int16 dma_gather range.
Only the SOURCE side is DMA-gathered (one gather per edge); the destination
transform xr[dst] is reconstructed on the PE via a one-hot matmul from an
SBUF-resident per-shard xr table, and the e = xl[src]+xr[dst] sum is also
accumulated on the PE (identity matmul) so the Vector engine never touches
PSUM for the add. One-hot matrices (both orientations) are built with
is_equal against step-1 constant iota tiles so the DVE runs in 2x packed
mode. Gather descriptor generation (the GpSimd bottleneck) is trimmed with
per-core runtime valid-edge counts (num_idxs_reg) so padding lanes are
skipped. j1/j2 and m1/m2 share edge streams (identical schedules).
"""
import os
import sys

sys.path.insert(0, '/opt/trn_rl_repo')

import numpy as np
import ml_dtypes
import concourse.bacc as bacc
import concourse.mybir as mybir
import concourse.tile as tile
from concourse.bass_utils import run_bass_kernel_spmd
from concourse.library_config import mlp

bf16 = mybir.dt.bfloat16
f32 = mybir.dt.float32
i16 = mybir.dt.int16
i32 = mybir.dt.int32
AT = mybir.AluOpType
AF = mybir.ActivationFunctionType

CH = 8          # subtiles per gather chunk (NIDX = 1024)
QUAD = 4        # subtiles per PSUM e-accumulation group
PADLOC = 200.0  # dst_local sentinel for pad edges
LGCLAMP = 30.0  # logit clamp before exp (stale-lane safety)


def _wrap16(a):
    """int16 stream -> dma_gather wrapped layout [128, n/16]."""
    n = a.shape[0]
    w = np.zeros((16, (n + 15) // 16), np.int16)
    w[np.arange(n) % 16, np.arange(n) // 16] = a
    return np.tile(w, (8, 1))


def preprocess_conv(src, dst, ncores, npc, npad, lo_ps, hi_ps):
    """Bucket edges per core/block/half, pad to subtiles, build shared
    schedule and per-core streams.

    Returns sched, ns_lo, ns_hi, per-core streams (src16, dloc col layout,
    dlocR row layout), and per-core per-(block,half) edge counts."""
    nb = npad // 128
    loc = src % npc
    isl = loc < lo_ps
    rows = np.where(isl, (src // npc) * lo_ps + loc,
                    (src // npc) * hi_ps + loc - lo_ps)
    percore = []
    for c in range(ncores):
        sel = (dst >= c * npc) & (dst < (c + 1) * npc)
        r, d, lo = rows[sel], dst[sel] - c * npc, isl[sel]
        blk = d // 128
        blocks = []
        for b in range(nb):
            m = blk == b
            blocks.append((r[m & lo], d[m & lo], r[m & ~lo], d[m & ~lo]))
        percore.append(blocks)
    sched = []
    for b in range(nb):
        nlo = max(len(percore[c][b][0]) for c in range(ncores))
        nhi = max(len(percore[c][b][2]) for c in range(ncores))
        nlo = (nlo + 127) // 128
        nhi = (nhi + 127) // 128
        if nlo + nhi == 0:
            nlo = 1
        sched.append((nlo, nhi))
    ns_lo = sum(s[0] for s in sched)
    ns_hi = sum(s[1] for s in sched)
    ns = ns_lo + ns_hi
    out = []
    for c in range(ncores):
        s16 = np.zeros(ns * 128, np.int64)
        s16n = np.full(ns * 128, -1, np.int64)  # -1 pads: gen skipped
        dl = np.full(ns * 128, PADLOC, np.float32)
        olo, ohi = 0, ns_lo
        for b in range(nb):
            rl, dloc_l, rh, dloc_h = percore[c][b]
            nlo, nhi = sched[b]
            e0 = olo * 128
            s16[e0:e0 + len(rl)] = rl
            s16n[e0:e0 + len(rl)] = rl
            dl[e0:e0 + len(rl)] = dloc_l % 128
            olo += nlo
            e0 = ohi * 128
            s16[e0:e0 + len(rh)] = rh
            s16n[e0:e0 + len(rh)] = rh
            dl[e0:e0 + len(rh)] = dloc_h % 128
            ohi += nhi
        dloc_col = dl.reshape(ns, 128).T.astype(ml_dtypes.bfloat16)
        out.append({
            "src16": _wrap16(s16.astype(np.int16)),
            "src16n": _wrap16(s16n.astype(np.int16)),
            "dloc": np.ascontiguousarray(dloc_col),
            "dlocR": dl[None, :].astype(ml_dtypes.bfloat16),
        })
    return sched, ns_lo, ns_hi, out


def chunk_plan(sched, ns_lo):
    """Block-aligned chunking: chunks never span dst blocks or halves.
    Returns cdef {(half,cid): (o0, nk, b)}, cmap {(half,ordinal):
    (cid, slot)}."""
    cmap, cdef = {}, {}
    for half in ("lo", "hi"):
        o, cid = 0, 0
        for b, (nlo, nhi) in enumerate(sched):
            n = nlo if half == "lo" else nhi
            for s0 in range(0, n, CH):
                nk = min(CH, n - s0)
                cdef[(half, cid)] = (o + s0, nk, b)
                for j in range(nk):
                    cmap[(half, o + s0 + j)] = (cid, j)
                cid += 1
            o += n
    return cdef, cmap


def aug_w(W, b):
    """[din, HC],[HC] -> bf16 [din+1, HC] with bias row appended."""
    return np.vstack([W, b[None, :]]).astype(ml_dtypes.bfloat16)


def build_program(ncores, npc, npad, lo_ps, hi_ps, scheds, plans):
    """Build the full Bass program.
    scheds: dict edgeset->(sched, ns_lo, ns_hi); plans: edgeset->(cdef,cmap,nch)
    Returns nc."""
    nb = npad // 128
    nbl = lo_ps // 128
    nc = bacc.Bacc("TRN2", target_bir_lowering=False, debug=False,
                   num_devices=ncores, num_swdge_queues=4)

    D1, D2 = 128, 256   # HC per layer
    xbt = nc.dram_tensor("xbt", [65, npad], bf16, kind="ExternalInput")
    iotin = nc.dram_tensor("iot", [1, 128], bf16, kind="ExternalInput")
    identin = nc.dram_tensor("ident", [128, 128], bf16, kind="ExternalInput")
    iotPin = nc.dram_tensor("iotP", [128, CH * 128], bf16,
                            kind="ExternalInput")
    epsin = nc.dram_tensor("epsin", [1, 2], f32, kind="ExternalInput")

    # shared per-edge-set streams (j: job edges, m: mac edges). The j set
    # has a pad-0 variant (conv j1 runs first, pool buffers uninitialized)
    # and a pad(-1) variant (descriptor gen skipped) for conv j2.
    estreams = {}
    for es in ("j", "m"):
        ns = scheds[es][1] + scheds[es][2]
        estreams[es] = {
            "dlocR": nc.dram_tensor(f"{es}_dlocR", [1, ns * 128], bf16,
                                    kind="ExternalInput"),
            "dloc": nc.dram_tensor(f"{es}_dloc", [128, ns], bf16,
                                   kind="ExternalInput"),
        }
    estreams["j"]["src16_a"] = nc.dram_tensor(
        "j_src16_a", [128, (scheds["j"][1] + scheds["j"][2]) * 8], i16,
        kind="ExternalInput")
    estreams["j"]["src16_b"] = nc.dram_tensor(
        "j_src16_b", [128, (scheds["j"][1] + scheds["j"][2]) * 8], i16,
        kind="ExternalInput")
    estreams["m"]["src16_b"] = nc.dram_tensor(
        "m_src16_b", [128, (scheds["m"][1] + scheds["m"][2]) * 8], i16,
        kind="ExternalInput")

    convs = {}
    for p, hc in [("j1", D1), ("m1", D1), ("j2", D2), ("m2", D2)]:
        t = {}
        t["att"] = nc.dram_tensor(f"{p}_att", [1, hc], bf16,
                                  kind="ExternalInput")
        t["wla"] = nc.dram_tensor(f"{p}_wla", [65, hc], bf16,
                                  kind="ExternalInput")
        t["wra"] = nc.dram_tensor(f"{p}_wra", [65, hc], bf16,
                                  kind="ExternalInput")
        if p in ("j2", "m2"):
            t["wlb"] = nc.dram_tensor(f"{p}_wlb", [128, hc], bf16,
                                      kind="ExternalInput")
            t["wrb"] = nc.dram_tensor(f"{p}_wrb", [128, hc], bf16,
                                      kind="ExternalInput")
            t["bias"] = nc.dram_tensor(f"{p}_bias", [1, 128], f32,
                                       kind="ExternalInput")
        else:
            t["bias"] = nc.dram_tensor(f"{p}_bias", [1, 64], f32,
                                       kind="ExternalInput")
        convs[p] = t
    outp = nc.dram_tensor("outp", [npad, 128], f32, kind="ExternalOutput")

    qc = [0]  # gather queue rotator

    with tile.TileContext(nc) as tc:
        with (
            tc.tile_pool(name="res", bufs=1) as rp,
            tc.tile_pool(name="stream", bufs=1) as sp,
            tc.tile_pool(name="gat", bufs=4) as gp,
            tc.tile_pool(name="wrk", bufs=2) as wp,
            tc.tile_pool(name="epi", bufs=2) as ep,
            tc.tile_pool(name="tb", bufs=2) as tbp,
            tc.tile_pool(name="pacc", bufs=2, space="PSUM") as pacc,
            tc.tile_pool(name="peps", bufs=2, space="PSUM") as peps,
            tc.tile_pool(name="ptb", bufs=1, space="PSUM") as ptb,
            tc.tile_pool(name="dram", bufs=1, space="DRAM") as dp,
        ):
            nc.gpsimd.load_library(mlp)
            xbt_t = rp.tile([65, npad], bf16)
            nc.sync.dma_start(xbt_t[:], xbt[:])
            iota_t = rp.tile([128, 128], bf16)
            nc.sync.dma_start(iota_t[:], iotin[:].to_broadcast((128, 128)))
            ident_t = rp.tile([128, 128], bf16)
            nc.sync.dma_start(ident_t[:], identin[:])
            iotP_t = rp.tile([128, CH * 128], bf16)
            nc.sync.dma_start(iotP_t[:], iotPin[:])
            eps_t = rp.tile([128, 2], f32)
            nc.sync.dma_start(eps_t[:], epsin[:].to_broadcast((128, 2)))
            h1T = rp.tile([128, npad], bf16)
            stash = rp.tile([128, npad], bf16)

            def load_streams(es, variant):
                """Load shared streams for edge set es; returns dict."""
                ns = scheds[es][1] + scheds[es][2]
                st = estreams[es]
                s16_t = sp.tile([128, ns * 8], i16, tag=f"s16_{es}_{variant}")
                nc.sync.dma_start(s16_t[:], st[f"src16_{variant}"][:])
                dloc_t = sp.tile([128, ns], bf16, tag=f"dloc_{es}")
                nc.sync.dma_start(dloc_t[:], st["dloc"][:])
                return {"s16": s16_t, "dlocR": st["dlocR"], "dloc": dloc_t}

            def build_tables(p, hc, with_h1):
                """Own-shard XL (dram, AllGathered) + resident SBUF XR table."""
                t = convs[p]
                wla_t = rp.tile([65, hc], bf16, tag=f"w_{p}l")
                nc.sync.dma_start(wla_t[:], t["wla"][:])
                wra_t = rp.tile([65, hc], bf16, tag=f"w_{p}r")
                nc.sync.dma_start(wra_t[:], t["wra"][:])
                if with_h1:
                    wlb_t = rp.tile([128, hc], bf16, tag=f"w_{p}lb")
                    nc.sync.dma_start(wlb_t[:], t["wlb"][:])
                    wrb_t = rp.tile([128, hc], bf16, tag=f"w_{p}rb")
                    nc.sync.dma_start(wrb_t[:], t["wrb"][:])
                xl_sh_lo = dp.tile([lo_ps, hc], bf16, tag=f"xlshl_{p}")
                xl_sh_hi = dp.tile([hi_ps, hc], bf16, tag=f"xlshh_{p}")
                xl_f_lo = dp.tile([ncores * lo_ps, hc], bf16, tag=f"xlfl_{p}")
                xl_f_hi = dp.tile([ncores * hi_ps, hc], bf16, tag=f"xlfh_{p}")
                # resident xr: [128, nb, D2] (L1 uses low hc cols)
                xr_t = rp.tile([128, nb, D2], bf16, tag=f"xr_{p[0]}")
                for side in (0, 1):
                    wa = wla_t if side == 0 else wra_t
                    for jt in range(nb):
                        ps = ptb.tile([128, hc], f32, space="PSUM", tag="tb")
                        nc.tensor.matmul(ps[:], lhsT=xbt_t[:, jt * 128:(jt + 1) * 128],
                                         rhs=wa[:], start=True, stop=not with_h1)
                        if with_h1:
                            wb = wlb_t if side == 0 else wrb_t
                            nc.tensor.matmul(ps[:],
                                             lhsT=h1T[:, jt * 128:(jt + 1) * 128],
                                             rhs=wb[:], start=False, stop=True)
                        if side == 0:
                            sb = tbp.tile([128, hc], bf16, tag="tbsb")
                            nc.scalar.copy(sb[:], ps[:])
                            if jt < nbl:
                                nc.sync.dma_start(
                                    xl_sh_lo[jt * 128:(jt + 1) * 128, :],
                                    sb[:])
                            else:
                                nc.sync.dma_start(
                                    xl_sh_hi[(jt - nbl) * 128:
                                             (jt - nbl + 1) * 128, :], sb[:])
                        else:
                            nc.scalar.copy(xr_t[:, jt, 0:hc], ps[:])
                    if side == 0:
                        nc.gpsimd.collective_compute(
                            "AllGather", AT.bypass,
                            replica_groups=[list(range(ncores))],
                            ins=[xl_sh_lo.opt()], outs=[xl_f_lo.opt()])
                        nc.gpsimd.collective_compute(
                            "AllGather", AT.bypass,
                            replica_groups=[list(range(ncores))],
                            ins=[xl_sh_hi.opt()], outs=[xl_f_hi.opt()])
                convs[p]["xlf_lo"] = xl_f_lo
                convs[p]["xlf_hi"] = xl_f_hi
                convs[p]["xr"] = xr_t

            def conv_pass(p, hc, epi, variant="b"):
                es = p[0]
                sched, ns_lo, ns_hi = scheds[es]
                cdef, cmap = plans[es]
                t = convs[p]
                xr_t = t["xr"]
                strm = load_streams(es, variant)
                s16_t = strm["s16"]
                dlocR, dlocC = strm["dlocR"], strm["dloc"]
                chc = hc // 2
                att_t = rp.tile([128, hc], bf16, tag=f"att_{p}")
                nc.sync.dma_start(att_t[:], t["att"][:].to_broadcast((128, hc)))
                bw = 64 if p in ("j1", "m1") else 128
                bias_t = rp.tile([128, bw], f32, tag=f"bias_{p}")
                nc.sync.dma_start(bias_t[:], t["bias"][:].to_broadcast((128, bw)))

                xl_lo = t["xlf_lo"][:, :]
                xl_hi = t["xlf_hi"][:, :]
                half_off = {"lo": 0, "hi": ns_lo}
                chunks = {}

                def get_chunk(half, k):
                    key = (half, k)
                    if key in chunks:
                        return chunks[key]
                    o0, nk, b = cdef[key]
                    g0 = half_off[half] + o0  # global subtile offset
                    nidx = nk * 128
                    gt = gp.tile([128, CH, hc], bf16, tag="gsrc")
                    nc.gpsimd.dma_gather(
                        gt[:, 0:nk, :], xl_lo if half == "lo" else xl_hi,
                        s16_t[:, g0 * 8:(g0 + nk) * 8], nidx, nidx, hc,
                        queue_num=qc[0] % 4)
                    qc[0] += 1
                    dbc = gp.tile([128, CH * 128], bf16, tag="dbc", bufs=3)
                    nc.sync.dma_start(
                        dbc[:, 0:nidx],
                        dlocR[0:1, g0 * 128:(g0 + nk) * 128]
                        .to_broadcast((128, nidx)))
                    oh = gp.tile([128, CH, 128], bf16, tag="oh")
                    nc.vector.tensor_tensor(
                        out=oh[:, 0:nk, :],
                        in0=dlocC[:, g0:g0 + nk]
                        .rearrange("p (a b) -> p a b", b=1)
                        .to_broadcast((128, nk, 128)),
                        in1=iota_t[:].rearrange("p (a c) -> p a c", a=1)
                        .to_broadcast((128, nk, 128)), op=AT.is_equal)
                    ohT = gp.tile([128, CH * 128], bf16, tag="ohT")
                    nc.vector.tensor_tensor(
                        out=ohT[:, 0:nidx], in0=iotP_t[:, 0:nidx],
                        in1=dbc[:, 0:nidx], op=AT.is_equal)
                    ea = gp.tile([128, CH, hc], bf16, tag="ea", bufs=2)
                    for q0 in range(0, nk, QUAD):
                        qn = min(QUAD, nk - q0)
                        eps = peps.tile([128, QUAD, hc], f32, space="PSUM",
                                        tag="eps")
                        for s in range(q0, q0 + qn):
                            nc.tensor.matmul(
                                eps[:, s - q0, :],
                                lhsT=ohT[:, s * 128:(s + 1) * 128],
                                rhs=xr_t[:, b, 0:hc], start=True, stop=False)
                            nc.tensor.matmul(
                                eps[:, s - q0, :], lhsT=ident_t[:],
                                rhs=gt[:, s, :], start=False, stop=True)
                        nc.scalar.activation(
                            ea[:, q0:q0 + qn, :], eps[:, 0:qn, :],
                            AF.Prelu, alpha=0.15)
                    es_t = wp.tile([128, CH, hc], bf16, tag="es")
                    nc.vector.tensor_tensor(
                        out=es_t[:, 0:nk, :], in0=ea[:, 0:nk, :],
                        in1=att_t[:].rearrange("p (a c) -> p a c", a=1)
                        .to_broadcast((128, nk, hc)), op=AT.mult)
                    hs_t = wp.tile([128, CH * 2, chc // 2], bf16, tag="hs")
                    ev = es_t[:, 0:nk, :].rearrange(
                        "p a (h g c) -> p (a h) g c", h=2, g=2)
                    nc.vector.tensor_tensor(
                        out=hs_t[:, 0:nk * 2, :],
                        in0=ev[:, :, 0, :], in1=ev[:, :, 1, :], op=AT.add)
                    lg = wp.tile([128, CH * 2], f32, tag="lg")
                    nc.vector.tensor_reduce(
                        out=lg[:, 0:nk * 2],
                        in_=hs_t[:, 0:nk * 2, :],
                        axis=mybir.AxisListType.X, op=AT.add)
                    gtw = wp.tile([128, CH, hc + 2], bf16, tag="gtw")
                    nc.scalar.activation(
                        gtw[:, 0:nk, hc:hc + 2],
                        lg[:, 0:nk * 2].rearrange("p (a h) -> p a h", h=2),
                        AF.Exp)
                    nc.vector.tensor_tensor(
                        out=gtw[:, 0:nk, 0:hc].rearrange(
                            "p a (h c) -> p a h c", h=2),
                        in0=gt[:, 0:nk, :].rearrange(
                            "p a (h c) -> p a h c", h=2),
                        in1=gtw[:, 0:nk, hc:hc + 2]
                        .rearrange("p a (h o) -> p a h o", o=1)
                        .to_broadcast((128, nk, 2, chc)), op=AT.mult)
                    ck = {"gtw": gtw, "oh": oh}
                    chunks[key] = ck
                    return ck

                olo, ohi = 0, 0
                for b in range(nb):
                    nlo, nhi = sched[b]
                    subs = ([("lo", olo + i) for i in range(nlo)]
                            + [("hi", ohi + i) for i in range(nhi)])
                    olo += nlo
                    ohi += nhi
                    acc = pacc.tile([128, hc + 2], f32, space="PSUM", tag="acc")
                    n = len(subs)
                    for si, (half, o) in enumerate(subs):
                        cid, s = cmap[(half, o)]
                        ck = get_chunk(half, cid)
                        nc.tensor.matmul(acc[:, 0:hc + 2],
                                         lhsT=ck["oh"][:, s, :],
                                         rhs=ck["gtw"][:, s, :],
                                         start=(si == 0), stop=(si == n - 1))
                    epi(b, acc, bias_t)

            def epi_l1(rowoff):
                def f(b, acc, bias_t):
                    cp = ep.tile([128, 130], f32, tag="cp")
                    nc.scalar.copy(cp[:], acc[:])
                    den = ep.tile([128, 2], f32, tag="den")
                    nc.vector.tensor_tensor(out=den[:], in0=cp[:, 128:130],
                                            in1=eps_t[:], op=AT.add)
                    nc.vector.reciprocal(den[:], den[:])
                    xt = ep.tile([128, 2, 64], f32, tag="xt")
                    nc.vector.tensor_tensor(
                        out=xt[:],
                        in0=cp[:, 0:128].rearrange("p (h c) -> p h c", h=2),
                        in1=den[:].rearrange("p (h o) -> p h o", o=1)
                        .to_broadcast((128, 2, 64)), op=AT.mult)
                    st = ep.tile([128, 64], f32, tag="st")
                    nc.vector.tensor_tensor(out=st[:], in0=xt[:, 0, :],
                                            in1=xt[:, 1, :], op=AT.add)
                    nc.vector.tensor_tensor(out=st[:], in0=st[:], in1=bias_t[:],
                                            op=AT.add)
                    hb = ep.tile([128, 64], bf16, tag="hb")
                    nc.scalar.activation(hb[:], st[:], AF.Relu, scale=0.5)
                    tp = ptb.tile([64, 128], bf16, space="PSUM", tag="tr")
                    nc.tensor.transpose(tp[:], hb[:], ident_t[:])
                    nc.vector.tensor_copy(
                        h1T[rowoff:rowoff + 64, b * 128:(b + 1) * 128], tp[:])
                return f

            def epi_l2(is_j):
                def f(b, acc, bias_t):
                    cp = ep.tile([128, 258], f32, tag="cp2")
                    nc.scalar.copy(cp[:], acc[:])
                    den = ep.tile([128, 2], f32, tag="den")
                    nc.vector.tensor_tensor(out=den[:], in0=cp[:, 256:258],
                                            in1=eps_t[:], op=AT.add)
                    nc.vector.reciprocal(den[:], den[:])
                    xt = ep.tile([128, 2, 128], f32, tag="xt2")
                    nc.vector.tensor_tensor(
                        out=xt[:],
                        in0=cp[:, 0:256].rearrange("p (h c) -> p h c", h=2),
                        in1=den[:].rearrange("p (h o) -> p h o", o=1)
                        .to_broadcast((128, 2, 128)), op=AT.mult)
                    st = ep.tile([128, 128], f32, tag="st2")
                    nc.vector.tensor_tensor(out=st[:], in0=xt[:, 0, :],
                                            in1=xt[:, 1, :], op=AT.add)
                    nc.vector.tensor_tensor(out=st[:], in0=st[:], in1=bias_t[:],
                                            op=AT.add)
                    if is_j:
                        nc.vector.tensor_copy(stash[:, b * 128:(b + 1) * 128],
                                              st[:])
                    else:
                        nc.vector.tensor_tensor(
                            out=st[:], in0=st[:],
                            in1=stash[:, b * 128:(b + 1) * 128], op=AT.add)
                        ob = ep.tile([128, 128], f32, tag="ob")
                        nc.scalar.activation(ob[:], st[:], AF.Relu, scale=0.25)
                        nc.sync.dma_start(outp[b * 128:(b + 1) * 128, :], ob[:])
                return f

            build_tables("j1", D1, False)
            conv_pass("j1", D1, epi_l1(0), variant="a")
            build_tables("m1", D1, False)
            conv_pass("m1", D1, epi_l1(64))
            build_tables("j2", D2, True)
            conv_pass("j2", D2, epi_l2(True), variant="a")
            build_tables("m2", D2, True)
            conv_pass("m2", D2, epi_l2(False))

    nc.compile()
    return nc


def run_full(inputs, N, ncores, npc=None):
    """Host orchestration: preprocess, build, run, assemble output."""
    x = np.asarray(inputs["x"], np.float32)
    npc = npc or N // ncores
    npad = ((npc + 127) // 128) * 128
    lo_ps = ((npad // 128 + 1) // 2) * 128
    hi_ps = npad - lo_ps
    IN = x.shape[1]

    def prep_edges(e):
        e = np.asarray(e)
        return e[0].astype(np.int64), e[1].astype(np.int64)

    js, jd = prep_edges(inputs["job_edges"])
    ms, md = prep_edges(inputs["mac_edges"])

    scheds, streams, plans = {}, {}, {}
    for es, (s, d) in [("j", (js, jd)), ("m", (ms, md))]:
        sch, nlo, nhi, pc = preprocess_conv(s, d, ncores, npc, npad,
                                            lo_ps, hi_ps)
        scheds[es] = (sch, nlo, nhi)
        streams[es] = pc
        plans[es] = chunk_plan(sch, nlo)

    nc = build_program(ncores, npc, npad, lo_ps, hi_ps, scheds, plans)

    # host-side constants
    xall = np.zeros((65, ncores * npad), np.float32)
    for c in range(ncores):
        n0 = c * npc
        w = min(npc, N - n0) if n0 < N else 0
        if w > 0:
            xall[0:IN, c * npad:c * npad + w] = x[n0:n0 + w].T
    xall[64, :] = 1.0
    xall = xall.astype(ml_dtypes.bfloat16)

    iot = np.arange(128, dtype=np.float32)[None, :].astype(ml_dtypes.bfloat16)
    ident = np.eye(128, dtype=np.float32).astype(ml_dtypes.bfloat16)
    iotP = np.tile(np.arange(128, dtype=np.float32)[:, None],
                   (1, CH * 128)).astype(ml_dtypes.bfloat16)

    consts = {}
    for p, key in [("j1", "jg1"), ("m1", "mg1"), ("j2", "jg2"), ("m2", "mg2")]:
        Wl = np.asarray(inputs[key + "_Wl"], np.float32)
        bl = np.asarray(inputs[key + "_bl"], np.float32)
        Wr = np.asarray(inputs[key + "_Wr"], np.float32)
        br = np.asarray(inputs[key + "_br"], np.float32)
        att = np.asarray(inputs[key + "_att"], np.float32)
        b = np.asarray(inputs[key + "_b"], np.float32)
        hc = Wl.shape[1]
        d = {}
        d[f"{p}_att"] = att.reshape(1, hc).astype(ml_dtypes.bfloat16)
        if p in ("j1", "m1"):
            d[f"{p}_wla"] = aug_w(Wl, bl)
            d[f"{p}_wra"] = aug_w(Wr, br)
            d[f"{p}_bias"] = (2.0 * b)[None, :].astype(np.float32)
        else:
            d[f"{p}_wla"] = aug_w(Wl[0:64], bl)
            d[f"{p}_wlb"] = Wl[64:192].astype(ml_dtypes.bfloat16)
            d[f"{p}_wra"] = aug_w(Wr[0:64], br)
            d[f"{p}_wrb"] = Wr[64:192].astype(ml_dtypes.bfloat16)
            d[f"{p}_bias"] = (2.0 * b)[None, :].astype(np.float32)
        consts.update(d)

    in_maps = []
    for c in range(ncores):
        m = {"xbt": xall[:, c * npad:(c + 1) * npad].copy(),
             "iot": iot, "ident": ident, "iotP": iotP,
             "epsin": np.full((1, 2), 1e-16, np.float32)}
        m.update(consts)
        stj, stm = streams["j"][c], streams["m"][c]
        m["j_src16_a"] = stj["src16"]
        m["j_src16_b"] = stj["src16"]
        m["j_dlocR"] = stj["dlocR"]
        m["j_dloc"] = stj["dloc"]
        m["m_src16_b"] = stm["src16"]
        m["m_dlocR"] = stm["dlocR"]
        m["m_dloc"] = stm["dloc"]
        in_maps.append(m)

    res = run_bass_kernel_spmd(nc, in_maps, list(range(ncores)),
                               trace=bool(int(os.environ.get(
                                   'GAT_TRACE', '0'))))
    parts = [res.results[c]["outp"][:min(npc, N - c * npc)]
             for c in range(ncores)]
    out = np.concatenate([x, np.vstack(parts)], axis=1).astype(np.float32)
    return out, res


def _install_profile_shim():
    """Optional: register the NTFF profiling hook (GAT_TRACE=1)."""
    try:
        import types
        import antenv
        if "antenv.axon_hooks" not in sys.modules:
            _store = {}
            m = types.ModuleType("antenv.axon_hooks")
            m.set_axon_ntff_profile_hook = lambda h: _store.__setitem__("h", h)
            m.get_axon_ntff_profile_hook = lambda: _store.get("h")
            sys.modules["antenv.axon_hooks"] = m
            antenv.axon_hooks = m
        from trn_agent_boot.trn_boot import _ntff_profile_via_ctypes
        sys.modules["antenv.axon_hooks"].set_axon_ntff_profile_hook(
            _ntff_profile_via_ctypes("/opt/axon/libaxon_pjrt.so"))
    except Exception:
        pass


LAST_RESULT = None


def kernel(**inputs):
    global LAST_RESULT
    if os.environ.get("GAT_TRACE", "0") == "1":
        _install_profile_shim()
    out, res = run_full(inputs, 50000, 8)
    LAST_RESULT = res
    return out
